# revision 10
# baseline (speedup 1.0000x reference)
"""TRN2 Bass kernel for nn_Attention_43963285242501 — fused single-pass design.

Sharding: 8 cores = (batch b in {0,1}) x (kv-head group g in {0..3}).
Each core computes, for its batch, the 8 query heads + 1 kv head of group g,
the matching 512-wide slices of the gate and of Wo's rows, producing a
partial [L, D] output; the host sums the 4 partials per batch.

Redesign vs the 434us two-phase baseline: the TRN2 PE clock ramps
0.65 -> 1.2 -> 2.4 GHz with ~3us of continuous execution, and the old
phase C (attention) was ACT-exp co-bound, so its PE ran at the mid
p-state.  This version fuses everything into one software-pipelined
stream so the PE never idles:

  chunk pipeline (512-wide position chunks j=0..3):
    P(j):  kv/q/gate projections, norm + rope, PE transposes into qkT
    A(j):  attention for chunk j (ST -> exp -> PV per 128-tile, causal)
    O(j):  fused o_proj + y store
  emission: P(0); then for j: A(j) interleaved (Bresenham) with
  P(j+1)+O(j-1); finally O(3).  All engines' queues are in-order, so
  emission order is the schedule; interleaved projection matmuls fill
  every PE gap that the exp latency would otherwise create.

Other changes vs baseline:
  - single ACT table load for the whole kernel (exp_and_others =
    {exp, square, tanh, copy}): sigmoid(z) -> 0.5*(1+tanh(z/2)) with the
    0.5 folded into Wo on the host; rsqrt for the RMS norms via DVE
    Newton iterations seeded from DVE reciprocal (no sqrt table);
    a dummy exp first forces the right table choice.
  - k is roped directly in transposed layout from the kv projection
    output (partition-stacked cos/sin tables), and k's 1/rms is applied
    as the exp's per-partition scale (scale = SCALE*rsqrt(ms_k[kpos])),
    so k never round-trips through natural layout.
  - norm/rope commute exploited (qn_w = kn_w = 1 in this problem).
  - PSUM: st (2 bufs x 2 banks) + pv (2 banks) + one shared 1-bank ring
    (q/kv/gate/o_proj psums + transposes) x2 = exactly 8 banks.
"""

import sys

sys.path.insert(0, "/opt/trn_rl_repo")

import numpy as np

import concourse.mybir as mybir
import concourse.tile as tile
from concourse import bacc
from concourse.bass_utils import run_bass_kernel_spmd
from concourse.masks import make_identity

F32 = mybir.dt.float32
FP16 = mybir.dt.float16
MULT = mybir.AluOpType.mult
ADD = mybir.AluOpType.add
SUB = mybir.AluOpType.subtract
AXX = mybir.AxisListType.X
AF = mybir.ActivationFunctionType

B, L, D = 2, 2048, 2048
H, HKV, HD = 32, 4, 64
NH = H // HKV            # q heads per core = 8
NPAIR = NH // 2          # head pairs = 4
P = 128
EPS = 1e-5
THETA = 10000.0
SCALE = HD ** -0.5
HALF = HD // 2


def build_core_kernel(Lk=L, Dk=D):
    LT = Lk // P         # 16 pos tiles
    KC = Dk // P         # 16 contraction chunks over D
    QC = Lk // 512       # 4 pos chunks
    KT = 512 // P        # 4 pos-tiles per chunk

    nc = bacc.Bacc()
    xt = nc.dram_tensor("xt", [Dk, Lk], FP16, kind="ExternalInput")
    wq = nc.dram_tensor("wq", [Dk, NH * HD], FP16, kind="ExternalInput")
    wkv = nc.dram_tensor("wkv", [Dk, 2 * HD], FP16, kind="ExternalInput")
    wg = nc.dram_tensor("wg", [Dk, NH * HD], FP16, kind="ExternalInput")
    wo = nc.dram_tensor("wo", [NH * HD, Dk], FP16, kind="ExternalInput")  # x0.5
    cos_d = nc.dram_tensor("cos", [Lk, HALF], FP16, kind="ExternalInput")
    sin_d = nc.dram_tensor("sin", [Lk, HALF], FP16, kind="ExternalInput")
    mask_d = nc.dram_tensor("mask", [P, P], FP16, kind="ExternalInput")
    y = nc.dram_tensor("y", [Lk, Dk], FP16, kind="ExternalOutput")

    xt_r = xt.rearrange("(ko ki) l -> ki ko l", ki=P)          # [128, KC, Lk]
    wq_r = wq.rearrange("(ko ki) m -> ki ko m", ki=P)          # [128, KC, 512]
    wkv_r = wkv.rearrange("(ko ki) m -> ki ko m", ki=P)        # [128, KC, 128]
    wg_r = wg.rearrange("(ko ki) m -> ki ko m", ki=P)
    wo_r = wo.rearrange("(jo ji) d -> ji jo d", ji=P)          # [128, 4, Dk]
    cos_r = cos_d.rearrange("(t p) c -> p t c", p=P)           # [128, LT, 32]
    sin_r = sin_d.rearrange("(t p) c -> p t c", p=P)
    y_r = y.rearrange("(t p) d -> p t d", p=P)                 # [128, LT, Dk]

    with tile.TileContext(nc) as tc:
        with (
            tc.tile_pool(name="persist", bufs=1) as persist,
            tc.tile_pool(name="xq", bufs=2) as xq_pool,
            tc.tile_pool(name="work", bufs=2) as work,
            tc.tile_pool(name="prp", bufs=4) as prp,
            tc.tile_pool(name="ogp", bufs=2) as ogp,
            tc.tile_pool(name="ypool", bufs=2) as ypool,
            tc.tile_pool(name="stp", bufs=2, space="PSUM") as stp,
            tc.tile_pool(name="pvp", bufs=1, space="PSUM") as pvp,
            tc.tile_pool(name="ringp", bufs=2, space="PSUM") as ringp,
        ):
            # ---------------- persistent SBUF ----------------
            qkT = persist.tile([HD, NH + 1, Lk], FP16)  # heads 0..7 qT, 8 kT
            v_sb = persist.tile([P, LT, P], FP16)       # v | ones | zero-pad
            gateT = persist.tile([P, NPAIR, Lk], FP16)  # tanh(z/2)
            wo_sb = persist.tile([P, NH * HD // P, Dk], FP16)
            wq_sb = persist.tile([P, KC, NH * HD], FP16)
            wkv_sb = persist.tile([P, KC, 2 * HD], FP16)
            wg_sb = persist.tile([P, KC, NH * HD], FP16)
            cs_sb = persist.tile([P, LT, HD], FP16)     # [cos|sin] per pos
            sc_sb = persist.tile([P, LT, HD], FP16)     # [sin|cos] per pos
            krs = persist.tile([P, LT], F32)            # SCALE/rms_k per pos
            mask_sb = persist.tile([P, P], FP16)
            identh = persist.tile([P, P], FP16)
            dume = persist.tile([P, 1], F32)

            nc.vector.memset(v_sb[:], 0.0)
            nc.vector.memset(v_sb[:, :, HD : HD + 1], 1.0)
            nc.vector.memset(dume[:], 0.0)
            # first activation = exp so the single table load picks an
            # exp-containing set; square/tanh/copy all live there too.
            nc.scalar.activation(out=dume[:], in_=dume[:], func=AF.Exp)
            make_identity(nc, identh[:])

            # weights + tables on the gpsimd DMA queue, priority order
            for c in range(0, KC, 4):
                nc.gpsimd.dma_start(wkv_sb[:, c : c + 4], wkv_r[:, c : c + 4])
                nc.gpsimd.dma_start(wq_sb[:, c : c + 4], wq_r[:, c : c + 4])
            nc.gpsimd.dma_start(
                cs_sb.rearrange("p t (h c) -> p t h c", h=2)[:, :, 0], cos_r)
            nc.gpsimd.dma_start(
                cs_sb.rearrange("p t (h c) -> p t h c", h=2)[:, :, 1], sin_r)
            nc.gpsimd.dma_start(
                sc_sb.rearrange("p t (h c) -> p t h c", h=2)[:, :, 0], sin_r)
            nc.gpsimd.dma_start(
                sc_sb.rearrange("p t (h c) -> p t h c", h=2)[:, :, 1], cos_r)
            nc.gpsimd.dma_start(mask_sb[:], mask_d[:, :])
            for c in range(0, KC, 4):
                nc.gpsimd.dma_start(wg_sb[:, c : c + 4], wg_r[:, c : c + 4])
            nc.gpsimd.dma_start(wo_sb[:], wo_r[:, :])

            xq_tiles = {}

            def load_xq(j, chunked=False):
                xt_q = xq_pool.tile([P, KC, 512], FP16, tag="xtq", name="xt_q")
                qs = slice(j * 512, (j + 1) * 512)
                if chunked:
                    for c in range(0, KC, 4):
                        nc.sync.dma_start(xt_q[:, c : c + 4], xt_r[:, c : c + 4, qs])
                else:
                    nc.sync.dma_start(xt_q[:], xt_r[:, :, qs])
                xq_tiles[j] = xt_q

            load_xq(0, chunked=True)
            load_xq(1)

            outg_reg = {}

            # ------------- P(j): projections / norm / rope -------------
            # NOTE: every unit that allocates a tile from the shared PSUM
            # ring ("r") emits ALL of that tile's accesses within the same
            # unit, so arbitrary unit interleaving can never reallocate a
            # ring slot while a tile is still mid-accumulation.
            def make_P_units(j):
                units = []
                xt_q = xq_tiles[j]
                qs512 = slice(j * 512, (j + 1) * 512)
                cst = {}

                def kvu(cst=cst, xt_q=xt_q):
                    kv_ps = ringp.tile([P, 512], F32, tag="r", name="kv_ps")
                    for kc in range(KC):
                        nc.tensor.matmul(
                            kv_ps[:], wkv_sb[:, kc], xt_q[:, kc],
                            start=(kc == 0), stop=(kc == KC - 1),
                            skip_group_check=True,
                        )
                    kvT_f = work.tile([P, 512], FP16, tag="kvtf",
                                      name="kvT_f")
                    nc.vector.tensor_copy(kvT_f[:], kv_ps[:])
                    cst["kvT_f"] = kvT_f

                units.append(kvu)

                tst = [dict() for _ in range(KT)]
                ss_st = {}

                def qu(t, cst=cst, xt_q=xt_q, j=j):
                    # q projection + sumsq + raw rope, all consuming q_ps
                    # in-unit so the ring slot is immediately recyclable
                    ts = tst[t]
                    q_ps = ringp.tile([P, NH, HD], F32, tag="r", name="q_ps")
                    tsl = slice(t * P, (t + 1) * P)
                    for kc in range(KC):
                        nc.tensor.matmul(
                            q_ps[:], xt_q[:, kc, tsl], wq_sb[:, kc],
                            start=(kc == 0), stop=(kc == KC - 1),
                            skip_group_check=True,
                        )
                    if t == 0:
                        ss_st["ss"] = work.tile([P, KT, NH + 1], F32,
                                                tag="ssc", name="ss_c")
                    ss = ss_st["ss"]
                    qsq = work.tile([P, NH, HD], FP16, tag="qsq", name="qsq")
                    nc.scalar.activation(out=qsq[:], in_=q_ps[:],
                                         func=AF.Square)
                    nc.vector.reduce_sum(out=ss[:, t, 0:NH], in_=qsq[:],
                                         axis=AXX)
                    pt = j * KT + t
                    qro = work.tile([P, NH, HD], FP16, tag="qro", bufs=4,
                                    name="qro")
                    _rope(nc, work, qro, q_ps[:], cs_sb[:, pt],
                          sc_sb[:, pt], NH)
                    ts["qro"] = qro

                def kvtu(t, cst=cst, j=j):
                    # kv natural for v + k sumsq + raw k rope (k's 1/rms is
                    # applied later as the exp's per-partition scale)
                    ts = tst[t]
                    pt = j * KT + t
                    trkv = ringp.tile([P, P], FP16, tag="r", name="trkv")
                    nc.tensor.transpose(trkv[:], cst["kvT_f"][:, t * P : (t + 1) * P],
                                        identh[:])
                    nc.vector.tensor_copy(v_sb[:, pt, 0:HD], trkv[:, HD:P])
                    ksq = work.tile([P, HD], FP16, tag="ksq", name="ksq")
                    nc.scalar.activation(
                        out=ksq[:], in_=trkv[:, 0:HD], func=AF.Square,
                        accum_out=ss_st["ss"][:, t, NH : NH + 1])
                    kro = work.tile([P, HD], FP16, tag="kro", bufs=2,
                                    name="kro")
                    _rope1(nc, work, kro, trkv[:, 0:HD], cs_sb[:, pt],
                           sc_sb[:, pt])
                    ts["kro"] = kro

                def ktu(t, j=j):
                    # kT transpose into qkT (self-contained ring unit)
                    ts = tst[t]
                    pt = j * KT + t
                    ktps = ringp.tile([HD, P], FP16, tag="r", name="ktps")
                    nc.tensor.transpose(ktps[:], ts["kro"][:], identh[:])
                    nc.vector.tensor_copy(qkT[:, NH, pt * P : (pt + 1) * P],
                                          ktps[:])

                def newtonu(j=j):
                    # rsqrt(ss/HD + eps) for all tiles of the chunk, 2 Newton
                    # iterations seeded from reciprocal
                    ss = ss_st["ss"]
                    m_ = work.tile([P, KT, NH + 1], F32, tag="nm", name="m_")
                    nc.vector.tensor_scalar(m_[:], ss[:], 1.0 / HD, EPS,
                                            MULT, ADD)
                    yv = work.tile([P, KT, NH + 1], F32, tag="nyv", name="yv")
                    nc.vector.reciprocal(yv[:], m_[:])
                    nc.vector.tensor_scalar(yv[:], yv[:], 0.5, 0.5, MULT, ADD)
                    aa = work.tile([P, KT, NH + 1], F32, tag="naa", name="aa")
                    for _ in range(2):
                        nc.vector.tensor_tensor(aa[:], yv[:], yv[:], MULT)
                        nc.vector.tensor_tensor(aa[:], aa[:], m_[:], MULT)
                        nc.vector.tensor_scalar(aa[:], aa[:], -0.5, 1.5,
                                                MULT, ADD)
                        nc.vector.tensor_tensor(yv[:], yv[:], aa[:], MULT)
                    nc.vector.tensor_scalar_mul(
                        krs[:, j * KT : (j + 1) * KT],
                        yv[:, :, NH], SCALE)
                    ss_st["yv"] = yv

                def scaleu(t, j=j):
                    ts = tst[t]
                    yv = ss_st["yv"]
                    nc.vector.tensor_tensor(
                        ts["qro"][:], ts["qro"][:],
                        yv[:, t, 0:NH, None].to_broadcast([P, NH, HD]), MULT)

                def transu(t, j=j):
                    ts = tst[t]
                    pt = j * KT + t
                    trq8 = ringp.tile([HD, NH, P], FP16, tag="r", name="trq8")
                    for h in range(NH):
                        nc.tensor.transpose(trq8[:, h], ts["qro"][:, h, :],
                                            identh[:])
                    nc.vector.tensor_copy(qkT[:, 0:NH, pt * P : (pt + 1) * P],
                                          trq8[:])

                for t in range(KT):
                    units.append(lambda t=t: qu(t))
                    units.append(lambda t=t: kvtu(t))
                    if t > 0:
                        units.append(lambda t=t - 1: ktu(t))
                units.append(lambda: ktu(KT - 1))
                units.append(newtonu)
                for t in range(KT):
                    units.append(lambda t=t: scaleu(t))

                def gu(jc, xt_q=xt_q, qs512=qs512):
                    g_ps = ringp.tile([P, 512], F32, tag="r", name="g_ps")
                    for kc in range(KC):
                        nc.tensor.matmul(
                            g_ps[:], wg_sb[:, kc, jc * P : (jc + 1) * P],
                            xt_q[:, kc],
                            start=(kc == 0), stop=(kc == KC - 1),
                            skip_group_check=True,
                        )
                    nc.scalar.activation(out=gateT[:, jc, qs512],
                                         in_=g_ps[:], func=AF.Tanh,
                                         scale=0.5)

                for jc in range(NPAIR):
                    units.append(lambda jc=jc: gu(jc))
                    # scatter the q transposes among the gate groups
                    units.append(lambda t=jc: transu(t))
                return units

            # ---------------- A(j): attention ----------------
            def make_A_steps(j):
                steps = []
                nkt = KT * (j + 1)
                qsl = slice(j * 512, (j + 1) * 512)

                def prelude(j=j):
                    outg_reg[j] = ogp.tile([P, NPAIR, 512], FP16, tag="ogj",
                                           name="outg")

                steps.append(prelude)
                for p in range(NPAIR):
                    hA, hB = 2 * p, 2 * p + 1
                    pst = {"prs": {}}

                    def do_pv(kt, pst=pst, j=j, nkt=nkt):
                        off = kt - KT * j
                        qlo = off * P if off > 0 else 0
                        pr = pst["prs"].pop(kt)
                        pv = pst["pv"]
                        nc.tensor.matmul(
                            pv[:, 0, qlo:512], v_sb[:, kt, :],
                            pr[:, 0, qlo:512],
                            start=(kt == 0), stop=(kt == nkt - 1),
                            skip_group_check=True,
                        )
                        nc.tensor.matmul(
                            pv[:, 1, qlo:512], v_sb[:, kt, :],
                            pr[:, 1, qlo:512],
                            start=(kt == 0), stop=(kt == nkt - 1),
                            skip_group_check=True,
                        )

                    def step(kt, pst=pst, j=j, nkt=nkt, hA=hA, hB=hB,
                             do_pv=do_pv):
                        if kt == 0:
                            pst["pv"] = pvp.tile([P, 2, 512], F32, tag="pv",
                                                 name="pv")
                        off = kt - KT * j
                        qlo = off * P if off > 0 else 0
                        ksl = slice(kt * P, (kt + 1) * P)
                        gqs = slice(j * 512 + qlo, (j + 1) * 512)
                        stt = stp.tile([P, 2, 512], F32, tag="st", name="stt")
                        nc.tensor.matmul(
                            stt[:, 0, qlo:512], qkT[:, NH, ksl],
                            qkT[:, hA, gqs],
                            start=True, stop=True, skip_group_check=True)
                        nc.tensor.matmul(
                            stt[:, 1, qlo:512], qkT[:, NH, ksl],
                            qkT[:, hB, gqs],
                            start=True, stop=True, skip_group_check=True)
                        pr = prp.tile([P, 2, 512], FP16, tag="pr", name="pr")
                        pst["prs"][kt] = pr
                        nc.scalar.activation(
                            out=pr[:, :, qlo:512], in_=stt[:, :, qlo:512],
                            func=AF.Exp, scale=krs[:, kt : kt + 1])
                        if off >= 0:
                            nc.vector.tensor_tensor(
                                pr[:, :, qlo : qlo + P],
                                pr[:, :, qlo : qlo + P],
                                mask_sb[:, None, :].to_broadcast([P, 2, P]),
                                MULT)
                        if kt >= 2:
                            do_pv(kt - 2)

                    for kt in range(nkt):
                        steps.append(lambda kt=kt, step=step: step(kt))

                    def tail(do_pv=do_pv, nkt=nkt):
                        do_pv(nkt - 2)
                        do_pv(nkt - 1)

                    steps.append(tail)

                    def norm_pair(p=p, pst=pst, j=j, qsl=qsl):
                        pv = pst["pv"]
                        sm = work.tile([1, 2, 512], F32, tag="sm",
                                       bufs=1, name="sm")
                        nc.vector.tensor_copy(sm[:], pv[HD : HD + 1, :, :])
                        rec = work.tile([1, 2, 512], F32, tag="rec",
                                        bufs=1, name="rec")
                        nc.vector.reciprocal_approx_fast(
                            out=rec[:], in_=sm[:])
                        rbgA = work.tile([HD, 512], F32, tag="rbgA",
                                         bufs=1, name="rbgA")
                        rbgB = work.tile([HD, 512], F32, tag="rbgB",
                                         bufs=1, name="rbgB")
                        nc.gpsimd.partition_broadcast(rbgA[:], rec[:, 0, :])
                        nc.gpsimd.partition_broadcast(rbgB[:], rec[:, 1, :])
                        og = work.tile([P, 512], F32, tag="og", name="og")
                        nc.vector.tensor_tensor(
                            og[0:HD, :], pv[0:HD, 0, :], rbgA[:], MULT)
                        nc.vector.tensor_tensor(
                            og[HD:P, :], pv[0:HD, 1, :], rbgB[:], MULT)
                        nc.vector.scalar_tensor_tensor(
                            outg_reg[j][:, p], gateT[:, p, qsl], 1.0, og[:],
                            ADD, MULT)

                    steps.append(norm_pair)
                return steps

            # ---------------- O(j): fused o_proj ----------------
            def make_O_units(j):
                units = []
                for t in range(KT):
                    qt = j * KT + t
                    for dc in range(Dk // 512):
                        def ou(t=t, dc=dc, j=j, qt=qt):
                            outg = outg_reg[j]
                            y_ps = ringp.tile([P, 512], F32, tag="r",
                                              name="y_ps")
                            tsl = slice(t * P, (t + 1) * P)
                            for jc in range(NPAIR):
                                nc.tensor.matmul(
                                    y_ps[:], outg[:, jc, tsl],
                                    wo_sb[:, jc, dc * 512 : (dc + 1) * 512],
                                    start=(jc == 0), stop=(jc == NPAIR - 1),
                                    skip_group_check=True,
                                )
                            y_sb = ypool.tile([P, 512], FP16, tag="ysb",
                                              bufs=4, name="y_sb")
                            dcs = slice(dc * 512, (dc + 1) * 512)
                            if dc % 2 == 0:
                                nc.vector.tensor_copy(y_sb[:], y_ps[:])
                            else:
                                nc.scalar.copy(out=y_sb[:], in_=y_ps[:])
                            nc.sync.dma_start(y_r[:, qt, dcs], y_sb[:])
                        units.append(ou)
                return units

            def merge_lists(a, b):
                if not a:
                    return list(b)
                if not b:
                    return list(a)
                out = []
                r = len(b) / len(a)
                acc = 0.0
                bi = 0
                for x in a:
                    out.append(x)
                    acc += r
                    while bi < len(b) and bi + 1 <= acc + 1e-9:
                        out.append(b[bi])
                        bi += 1
                out.extend(b[bi:])
                return out

            def emit_interleaved(steps, filler):
                if not steps:
                    for f in filler:
                        f()
                    return
                r = len(filler) / len(steps)
                acc = 0.0
                fi = 0
                for s in steps:
                    s()
                    acc += r
                    while fi < len(filler) and fi + 1 <= acc + 1e-9:
                        filler[fi]()
                        fi += 1
                while fi < len(filler):
                    filler[fi]()
                    fi += 1

            # ---------------- main emission ----------------
            for f in make_P_units(0):
                f()
            for j in range(QC):
                if j + 2 < QC:
                    load_xq(j + 2)
                pl = make_P_units(j + 1) if j + 1 < QC else []
                ol = make_O_units(j - 1) if j - 1 >= 0 else []
                emit_interleaved(make_A_steps(j), merge_lists(pl, ol))
            for f in make_O_units(QC - 1):
                f()

    nc.compile()
    return nc


def _rope1(nc, pool, out, in_, cs_t, sc_t):
    """nh=1 rope for the k head: halves along the free dim."""
    ta = pool.tile([P, HD], F32, tag="rope1_a", name="rope1_a")
    tb = pool.tile([P, HD], F32, tag="rope1_b", name="rope1_b")
    nc.vector.tensor_tensor(ta[:], in_, cs_t, MULT)
    nc.vector.tensor_tensor(tb[:], in_, sc_t, MULT)
    nc.vector.tensor_tensor(out[:, 0:HALF], ta[:, 0:HALF], ta[:, HALF:HD], SUB)
    nc.vector.tensor_tensor(out[:, HALF:HD], tb[:, 0:HALF], tb[:, HALF:HD], ADD)


def _rope(nc, pool, out, in_, cs_t, sc_t, nh):
    """Split-half rope via packed tables: cs = [cos|sin], sc = [sin|cos].
    ta = in*cs = [x1*cos | x2*sin]; tb = in*sc = [x1*sin | x2*cos];
    out1 = ta1 - ta2; out2 = tb1 + tb2. 4 DVE ops."""
    o1 = out[:, :, 0:HALF]
    o2 = out[:, :, HALF:HD]
    csb = cs_t[:, None, :].to_broadcast([P, nh, HD])
    scb = sc_t[:, None, :].to_broadcast([P, nh, HD])
    shape = [P, nh, HD]

    def half(t, i):
        return t[:, :, i * HALF : (i + 1) * HALF]

    ta = pool.tile(shape, F32, tag="rope_a", name="rope_a")
    tb = pool.tile(shape, F32, tag="rope_b", name="rope_b")
    nc.vector.tensor_tensor(ta[:], in_, csb, MULT)
    nc.vector.tensor_tensor(tb[:], in_, scb, MULT)
    nc.vector.tensor_tensor(o1, half(ta, 0), half(ta, 1), SUB)
    nc.vector.tensor_tensor(o2, half(tb, 0), half(tb, 1), ADD)


def _host_inputs(x, Wq, Wk, Wv, Wg, Wo, Lk=L, Dk=D):
    """Build the 8 per-core input maps."""
    inv_freq = 1.0 / (THETA ** (np.arange(0, HALF, dtype=np.float64) / HALF))
    ang = np.arange(Lk, dtype=np.float64)[:, None] * inv_freq[None, :]
    cos_t = np.cos(ang).astype(np.float16)                      # [L, 32]
    sin_t = np.sin(ang).astype(np.float16)

    kk = np.arange(P)[:, None]
    qq = np.arange(P)[None, :]
    mask = (qq >= kk).astype(np.float16)

    in_maps = []
    for c in range(8):
        b, g = c // 4, c % 4
        xT = np.ascontiguousarray(x[b].T)
        in_maps.append(
            {
                "xt": xT.astype(np.float16),
                "wq": np.ascontiguousarray(Wq[:, g * NH * HD : (g + 1) * NH * HD]).astype(np.float16),
                "wkv": np.ascontiguousarray(
                    np.concatenate(
                        [
                            Wk[:, g * HD : (g + 1) * HD],
                            Wv[:, g * HD : (g + 1) * HD],
                        ],
                        axis=1,
                    )
                ).astype(np.float16),
                "wg": np.ascontiguousarray(Wg[:, g * NH * HD : (g + 1) * NH * HD]).astype(np.float16),
                "wo": (0.5 * np.ascontiguousarray(Wo[g * NH * HD : (g + 1) * NH * HD, :])).astype(np.float16),
                "cos": cos_t,
                "sin": sin_t,
                "mask": mask,
            }
        )
    return in_maps


_CACHED = {}


def kernel(x, Wq, Wk, Wv, Wg, Wo, qn_w, kn_w, mask, _trace=False, _tmpdir=None):
    """Full-input entry point. Returns [B, L, D] float32."""
    if "nc" not in _CACHED:
        _CACHED["nc"] = build_core_kernel()
    nc = _CACHED["nc"]
    in_maps = _host_inputs(
        np.asarray(x), np.asarray(Wq), np.asarray(Wk), np.asarray(Wv),
        np.asarray(Wg), np.asarray(Wo),
    )
    res = run_bass_kernel_spmd(
        nc, in_maps, core_ids=list(range(8)), trace=_trace, tmpdir=_tmpdir
    )
    out = np.zeros((B, L, D), dtype=np.float32)
    for c in range(8):
        out[c // 4] += res.results[c]["y"].astype(np.float32)
    if _trace:
        kernel.last_exec_time_ns = res.exec_time_ns
    return out
